# revision 63
# baseline (speedup 1.0000x reference)
"""MoE routing kernel for 8 Trainium2 NeuronCores.

Problem: B=65536 tokens, shared Linear(512->256)+ReLU, then per-token expert
MLP Linear(256->100)+ReLU -> Linear(100->1), expert chosen by idx in [0,16).

Strategy (expert-parallel, host-side routing):
  - Host sorts tokens by expert. Experts 2c and 2c+1 go to core c, each in a
    fixed-capacity slot (CA/CB = per-half max expert count rounded up to 8),
    padded with token 0 (padding outputs are computed then discarded).
  - x ships as fp8-e3m4 (x*2, with Ws/2 folded on the host): the PE accepts a
    mixed fp8-moving x bf16-stationary matmul at full bf16 rate, so only the
    x DMA bytes halve (rel err ~1.4e-2 vs the 2e-2 gate; fp8 weights or
    e4m3 anywhere fail the gate). xg is column-compacted on the host (each
    exec group packed densely at a prefix-sum offset) so every DMA moves
    contiguous >=512B per-partition rows.
  - Device, per group of <=512 tokens: 8 accumulating layer-1 matmuls
    (512-dim contraction, 2 hid chunks) + bias/ReLU (Vector/Scalar), 2
    matmuls for expert FC1 + bias/ReLU, 1 matmul for FC2 (b2 folded via a
    ones row of h1; stationary zero-padded to 128 cols so the PE tile
    config never changes). FOUR groups' FC2 rows accumulate into one PSUM
    bank at partitions 0/32/64/96, then one copy + one DMA out per block.
  - FOUR-deep software pipeline: iteration i runs L1(i), fc1(i-2), fc2(i-4),
    so every ReLU has ~2 groups of slack before its consuming matmul - one
    group is not enough during the small ramp groups, and any PE gap also
    defers the clock un-throttle (below) by a whole ~3.4us HAM window.
  - Clock: the PE boots throttled to 1.2GHz and un-throttles at a
    free-running ~3.4us HAM-window boundary after a fully-busy window.
    7 warm-up matmuls (never read, uninitialized operands) + the gap-free
    early stream keep it busy from program start; the warm-ups end right as
    the first x lands.
  - DMA: everything bulky rides the sync HWDGE ring in exact need-order
    (ws-hc1, x1, w1a, x2, x3, then PAIRS of groups per post - each post
    costs ~650ns of Sync time, which would otherwise cap the early supply
    rate; w1b drips in late). ONLY ws-hc0 + x0 ride the scalar ring, whose
    first packets arrive ~0.4us earlier - anything later there starves once
    the sync backlog builds. Tiny biases ride the gpsimd SWDGE. A supply
    ramp [128,128,256,256,256] opens slot A so the PE falls safely behind
    the DMA frontier.
  - Tail: slot B is carved so the LAST FOUR groups are [256,256,256,128]
    sharing one PSUM bank: the program ends with one short copy + one small
    DMA + the fixed ~1.6us HBM write receipt, before the NEFF epilogue
    (~6us of fixed ucode semaphore zeroing, outside our control but inside
    the measured window). A small keep-alive DMA keeps the sync ring hot
    for the final post (it also writes warm_w for the allocator).
  - Weights (tiny) live resident in SBUF in bf16; PSUM accumulates fp32.
"""

import math
import os
import sys

import numpy as np

for _p in ("/opt/trn_rl_repo", "/opt/pypackages"):
    if _p not in sys.path and os.path.isdir(_p):
        sys.path.append(_p)

import ml_dtypes

BF16 = ml_dtypes.bfloat16
F8E3 = ml_dtypes.float8_e3m4
X_SCALE = 2.0  # x*2 / Ws/2: shifts small |x| out of the e3m4 subnormal range

B, IN_DIM, HID, EXP_HID, OUT_DIM, N_EXP = 65536, 512, 256, 100, 1, 16
N_CORES = 8
GROUP = 512  # tokens per matmul group (= PSUM bank free-dim in fp32)
N_WARM = 7  # warm-up matmuls (~0.43us each cold), ending ~when the first x
# lands. The PE-clock HAM un-throttles at a free-running ~3.4us window
# boundary once it sees a fully-busy window: warm-ups + a GAP-FREE early
# real stream together cover the worst-case ~6.8us, so the clock warms as
# early as the phase allows (any early gap defers it by a whole window).

_PROGRAM_CACHE = {}


def _block_schedule(CA: int, CB: int):
    """Execution-order blocks: (exec_idx, expert_slot, token_offset, ntok).

    Slot A opens with a supply ramp over its first 1024 tokens (the DMA
    backlog is still building; the PE must fall safely behind the DMA
    frontier before group sizes reach 512 so the early stream is GAP-FREE
    for the clock-warming HAM window), then its full groups and remainder;
    slot B's full groups follow, with its tail carved into FOUR final
    groups [p, q, r, 128] that form the last 4-lane fc2 block (one copy +
    one small DMA at program end)."""
    n_full_b, r_b = divmod(CB, GROUP)
    if CA >= 2048:
        # = 4 full groups of ramp: PE demand stays ~1us behind the DMA
        # frontier through the whole clock-warming window
        a = [128, 128, 256, 256, 256, 384, 384, 256]
        rem = CA - 2048
    else:
        a = [128, 128, 256, 256, 256]
        rem = CA - 1024
    a += [GROUP] * (rem // GROUP)
    if rem % GROUP:
        a.append(rem % GROUP)
    b = [GROUP] * n_full_b
    if r_b:
        b.append(r_b)
    # carve the tail so the final four groups are ALWAYS [256, 256, 256, 128]
    # (the last 4-lane fc2 block: one short copy + one small DMA at program
    # end); the carve overflow becomes a regular group before them
    T = 0
    while b and T < 896:
        T += b.pop()
    if T > 896:
        b.append(T - 896)
    b += [256, 256, 256, 128]
    blocks = [(0, n) for n in a] + [(1, n) for n in b]
    out = []
    off = [0, 0]
    base = [0, CA]
    for slot, n in blocks:
        out.append((len(out), slot, base[slot] + off[slot], n))
        off[slot] += n
    return out


def _fc2_blocks(n_groups: int):
    """fc2 PSUM-sharing blocks: chunks of FOUR exec-groups (output lanes at
    partitions 0/32/64/96 of one PSUM bank — the 128-wide zero-padded
    stationary makes a 4th lane free). Built from the END so the final
    block is exactly the four small carved tail groups: the program ends
    with one (engine-split) copy + ONE small DMA; the HBM write receipt
    dominates the close."""
    head = n_groups - 4
    first = head % 4
    blocks = [list(range(first))] if first else []
    blocks += [list(range(i, i + 4)) for i in range(first, head, 4)]
    blocks.append([head, head + 1, head + 2, head + 3])
    return blocks


def _build_program(CA: int, CB: int):
    """Build (and cache) the Bass program for slot capacities (CA, CB)."""
    import concourse.bass as bass
    import concourse.mybir as mybir
    import concourse.tile as tile
    from concourse import bacc

    f32 = mybir.dt.float32
    bf16 = mybir.dt.bfloat16
    f8e3 = mybir.dt.float8e3
    AF = mybir.ActivationFunctionType
    ALU = mybir.AluOpType

    nc = bacc.Bacc("TRN2", target_bir_lowering=False, debug=False)

    groups = _block_schedule(CA, CB)
    n_groups = len(groups)
    fc2_blocks = _fc2_blocks(n_groups)
    n_blk = len(fc2_blocks)
    blk_of = {}
    blk_max = {}
    for bi, js in enumerate(fc2_blocks):
        blk_max[bi] = max(groups[j][3] for j in js)
        for lane, j in enumerate(js):
            blk_of[j] = (bi, lane, lane == len(js) - 1)

    # x pre-gathered on host in EXECUTION order, column-compacted fp8:
    # group g occupies cols [4*pre[g], 4*pre[g]+4*n) as [p, kc, t]
    pre = np.concatenate([[0], np.cumsum([n for _, _, _, n in groups])])
    tot_cols = 4 * int(pre[-1])
    xg_d = nc.dram_tensor("xg", [128, tot_cols], f8e3, kind="ExternalInput").ap()
    # ws packed p-major with contiguous hc halves: ws[p, hc, kc, m]
    ws_d = nc.dram_tensor("ws", [128, 2 * 4 * 128], bf16, kind="ExternalInput").ap()
    bs_d = nc.dram_tensor("bs", [128, 2], f32, kind="ExternalInput").ap()
    # w1 padded to 128 output cols (100 real) so FWL kicks in on LDWEIGHTS
    w1_d = nc.dram_tensor("w1", [128, 2 * 2 * 128], bf16, kind="ExternalInput").ap()
    # only the real fc2 columns come from DRAM (2KB); the mostly-zero
    # 128-wide stationary tiles are assembled on-chip
    w2c_d = nc.dram_tensor("w2c", [128, 2 * 4], bf16, kind="ExternalInput").ap()
    # b1 rows 0..99 = b1[e]; rows 100..127 = 1.0 so relu(0 + 1) makes a ones
    # row block that w2's bias row consumes (fc2 bias folded into the matmul)
    b1_d = nc.dram_tensor("b1", [128, 2], f32, kind="ExternalInput").ap()
    # out[blk, lane, t]: lanes hold the block's exec-groups' fc2 rows
    out_d = nc.dram_tensor("out", [n_blk, 4, GROUP], f32, kind="ExternalOutput").ap()

    with tile.TileContext(nc) as tc:
        with (
            tc.tile_pool(name="const", bufs=1) as const,
            tc.tile_pool(name="xp", bufs=1) as xp,
            tc.tile_pool(name="hp", bufs=5) as hp,
            tc.tile_pool(name="h1p", bufs=5) as h1p,
            tc.tile_pool(name="ob", bufs=3) as obp,
            tc.tile_pool(name="ps1", bufs=4, space="PSUM") as ps1,
            tc.tile_pool(name="ps2", bufs=2, space="PSUM") as ps2,
            tc.tile_pool(name="ps3", bufs=2, space="PSUM") as ps3,
        ):
            ws_sb = const.tile([128, 2, 4, 128], bf16)
            bs_sb = const.tile([128, 2], f32)
            w1_sb = const.tile([128, 2, 2, 128], bf16)
            b1_sb = const.tile([128, 2], f32)
            w2_sb = const.tile([128, 2, 4, 128], bf16)
            w2c_sb = const.tile([128, 2, 4], bf16)
            warm_w = const.tile([128, GROUP], f8e3)
            # x tiles are FLAT, dense, matching the DRAM layout, so src AND
            # dst of every x DMA are contiguous per-partition rows (>=512B
            # packets; a strided dst would split partial groups into
            # sub-512B packets, ~4x slower). From group 4 on, TWO adjacent
            # groups share one tile and ship in ONE DMA post: each post
            # costs ~650ns of Sync time, so per-group posting would cap the
            # supply rate (~260KB/650ns ~ 400GB/s) during the early build.
            # L1 slices kc chunks as [:, base + kc*n : base + (kc+1)*n].
            x_tiles = {}  # group -> (tile, base_col)
            post_units = []  # (first_group, src_lo, src_hi, tile, width)
            gi = 0
            while gi < n_groups:
                n1 = groups[gi][3]
                if gi < 4 or gi == n_groups - 1:
                    t = xp.tile([128, 4 * n1], f8e3, tag=f"x{gi}", name=f"x_sb{gi}")
                    x_tiles[gi] = (t, 0)
                    post_units.append((gi, 4 * int(pre[gi]), 4 * int(pre[gi + 1]), t))
                    gi += 1
                else:
                    n2 = groups[gi + 1][3]
                    t = xp.tile(
                        [128, 4 * (n1 + n2)], f8e3, tag=f"x{gi}", name=f"x_sb{gi}"
                    )
                    x_tiles[gi] = (t, 0)
                    x_tiles[gi + 1] = (t, 4 * n1)
                    post_units.append((gi, 4 * int(pre[gi]), 4 * int(pre[gi + 2]), t))
                    gi += 2

            unit_of = {u[0]: k for k, u in enumerate(post_units)}

            def post_x(u, eng=None):
                g0, lo, hi, t = post_units[u]
                (eng or nc.sync).dma_start(t[:, : hi - lo], xg_d[:, lo:hi])

            # dependency-free memset first: it runs during the framework
            # preamble window, long before the first ReLU needs Vector
            nc.vector.memset(w2_sb.rearrange("p e l m -> p (e l m)"), 0.0)

            # EVERYTHING bulky rides the sync ring in exact need-order (two
            # rings starve each other: once the sync ring's packet backlog
            # builds, the scalar ring drops to ~60GB/s and whatever is on it
            # arrives microseconds late). Tiny biases ride the gpsimd SWDGE.
            ws_dv = ws_d.rearrange("p (h k m) -> p h k m", h=2, k=4)
            w1_dv = w1_d.rearrange("p (e x) -> p e x", e=2)
            # the two transfers that gate the FIRST real matmul ride the
            # scalar ring: the Scalar engine exits the preamble ~0.4us
            # before Sync, so its ring spins up earlier — and both complete
            # before the sync ring's backlog could starve the scalar ring
            # only these two ride the scalar ring: they complete BEFORE the
            # sync ring's backlog builds — anything later on the scalar ring
            # starves (SDMA round-robin favors the deep sync ring)
            nc.scalar.dma_start(ws_sb[:, 0], ws_dv[:, 0])
            post_x(0, eng=nc.scalar)
            nc.sync.dma_start(ws_sb[:, 1], ws_dv[:, 1])
            post_x(1)
            nc.sync.dma_start(w1_sb[:, 0:1, :, :], w1_dv[:, 0:1, :])
            nc.gpsimd.dma_start(bs_sb[:, :], bs_d[:, :])
            nc.gpsimd.dma_start(b1_sb[:, :], b1_d[:, :])
            nc.gpsimd.dma_start(w2c_sb[:, :, :], w2c_d)
            # slot B's fc1 weights are first needed when slot B's first group
            # reaches fc1 — keep them out of the contested early window
            first_b = next(g for g, (_, s, _, _) in enumerate(groups) if s == 1)
            w1b_unit = unit_of[min(8, first_b - 1)] if min(8, first_b - 1) in unit_of \
                else unit_of[min(8, first_b - 1) - 1]
            for u in range(2, len(post_units)):
                post_x(u)
                if u == w1b_unit:
                    nc.sync.dma_start(w1_sb[:, 1:2, :, :], w1_dv[:, 1:2, :])

            # PE warm-up: full-width matmuls that keep the PE continuously
            # busy from program start until the first x tile lands, so the
            # PE clock (p-state) is fully ramped when real work begins. An
            # idle PE drops to half speed. Results are never read; warm_w is
            # deliberately uninitialized (garbage is harmless and skipping
            # the memset removes any vector-engine dependency).
            warm_p = ps1.tile([128, GROUP], f32, tag="p1", name="warm_p")
            for _ in range(N_WARM):
                nc.tensor.matmul(
                    warm_p[:, :], warm_w[:, :128], warm_w[:, :], start=True, stop=True
                )

            # assemble the zero-padded 128-wide fc2 stationary tiles: only
            # expert A's lanes are needed for the first fc2 blocks — its
            # copies go here; expert B's are deferred into the pipeline so
            # Scalar is free for the first hc1-ReLUs (the memset was issued
            # at body start so Vector is free too)
            def w2_assemble(e_):
                for lane_ in range(4):
                    nc.scalar.copy(
                        w2_sb[:, e_, lane_, 32 * lane_ : 32 * lane_ + 1],
                        w2c_sb[:, e_, lane_ : lane_ + 1],
                    )

            w2_assemble(0)
            w2_assemble(1)

            # Two-deep software pipeline over the PE stream: iteration i runs
            # L1(i), fc1(i-1), fc2(i-2). Every activation (h, h1) then has
            # >=1.5us between its producing ReLU and its consuming matmul, so
            # the in-order PE stream never stalls on the Vector/Scalar
            # engines.
            h_tiles = {}
            h1_tiles = {}
            p3 = None

            def do_l1(i):
                _, _, _, n = groups[i]
                x_sb, xb = x_tiles[i]
                h_sb = hp.tile([128, 2, GROUP], bf16, tag="h")
                for hc in range(2):
                    p1 = ps1.tile([128, GROUP], f32, tag="p1")
                    for kc in range(4):
                        nc.tensor.matmul(
                            p1[:, :n],
                            ws_sb[:, hc, kc, :],
                            x_sb[:, xb + kc * n : xb + (kc + 1) * n],
                            start=(kc == 0),
                            stop=(kc == 3),
                        )
                    # h = relu(psum + bs): hc0 on VectorE, hc1 on ScalarE
                    if hc == 0:
                        nc.vector.tensor_scalar(
                            h_sb[:, hc, :n],
                            p1[:, :n],
                            bs_sb[:, hc : hc + 1],
                            0.0,
                            ALU.add,
                            ALU.max,
                        )
                    else:
                        nc.scalar.activation(
                            h_sb[:, hc, :n],
                            p1[:, :n],
                            AF.Relu,
                            bias=bs_sb[:, hc : hc + 1],
                        )
                h_tiles[i] = h_sb

            def do_fc1(j):
                if j < 0 or j >= n_groups:
                    return
                _, e, _, n = groups[j]
                h_sb = h_tiles.pop(j)
                p2 = ps2.tile([128, GROUP], f32, tag="p2")
                for kc in range(2):
                    nc.tensor.matmul(
                        p2[:, :n],
                        w1_sb[:, e, kc, :],
                        h_sb[:, kc, :n],
                        start=(kc == 0),
                        stop=(kc == 1),
                    )
                # h1 rows 0..99 = relu(psum + b1); rows 100..127 = relu(0+1)=1
                # alternate engines so neither Vector nor Scalar is loaded 2x
                h1_sb = h1p.tile([128, GROUP], bf16, tag="h1")
                if j % 2 == 0:
                    nc.vector.tensor_scalar(
                        h1_sb[:, :n],
                        p2[:, :n],
                        b1_sb[:, e : e + 1],
                        0.0,
                        ALU.add,
                        ALU.max,
                    )
                else:
                    nc.scalar.activation(
                        h1_sb[:, :n],
                        p2[:, :n],
                        AF.Relu,
                        bias=b1_sb[:, e : e + 1],
                    )
                h1_tiles[j] = h1_sb

            def issue_fc2(j):
                nonlocal p3
                if j < 0 or j >= n_groups:
                    return
                _, e, _, n = groups[j]
                # up to 3 exec-groups share one PSUM bank at partitions
                # 0/32/64 (PE col-tile base must be 0/32/64)
                blk, lane, last = blk_of[j]
                if lane == 0:
                    p3 = ps3.tile([128, GROUP], f32, tag="p3")
                nc.tensor.matmul(
                    p3[:, :n],
                    w2_sb[:, e, lane, :],
                    h1_tiles.pop(j)[:, :n],
                    start=(lane == 0),
                    stop=last,
                )
                if last:
                    ob = obp.tile([128, GROUP], f32, tag="ob")
                    nrows = 32 * lane + 1
                    ncols = blk_max[blk]
                    if blk % 2 == 0:
                        nc.scalar.copy(ob[:nrows, :ncols], p3[:nrows, :ncols])
                    else:
                        nc.vector.tensor_scalar(
                            ob[:nrows, :ncols],
                            p3[:nrows, :ncols],
                            0.0,
                            0.0,
                            ALU.add,
                            ALU.bypass,
                        )
                    # rows 0/32/64 of ob hold the lanes' fc2 outputs
                    ob_lanes = ob.rearrange("(l s) t -> l s t", s=32)[:, 0, :]
                    nc.sync.dma_start(
                        out_d[blk, : lane + 1, :ncols], ob_lanes[: lane + 1, :ncols]
                    )

            # FOUR-deep software pipeline: iteration i runs L1(i), fc1(i-2),
            # fc2(i-4): every ReLU gets ~2 iterations of slack before its
            # consuming matmul (1 iteration is not enough during the small
            # ramp groups, where an h/h1 stall also re-throttles the PE clock)
            for i in range(n_groups):
                do_l1(i)
                do_fc1(i - 2)
                issue_fc2(i - 4)
            do_fc1(n_groups - 2)
            do_fc1(n_groups - 1)
            issue_fc2(n_groups - 4)
            issue_fc2(n_groups - 3)
            # small keep-alive so the sync ring is hot for the final posts
            # (also the write that lets the allocator place warm_w); it sits
            # one slot earlier in the ring FIFO so the final out post's path
            # is clean
            nc.sync.dma_start(warm_w[:, :], xg_d[:, :GROUP])
            issue_fc2(n_groups - 2)
            issue_fc2(n_groups - 1)

    nc.compile()
    return nc


def _get_program(CA: int, CB: int):
    if (CA, CB) not in _PROGRAM_CACHE:
        _PROGRAM_CACHE[(CA, CB)] = _build_program(CA, CB)
    return _PROGRAM_CACHE[(CA, CB)]


def kernel(x, idx, Ws, bs, W1, b1, W2, b2, _trace=False, _result_box=None):
    from concourse.bass_utils import run_bass_kernel_spmd

    x = np.asarray(x)
    idx = np.asarray(idx).astype(np.int64)
    Ws = np.asarray(Ws, dtype=np.float32)
    bs = np.asarray(bs, dtype=np.float32)
    W1 = np.asarray(W1, dtype=np.float32)
    b1 = np.asarray(b1, dtype=np.float32)
    W2 = np.asarray(W2, dtype=np.float32)
    b2 = np.asarray(b2, dtype=np.float32)

    counts = np.bincount(idx, minlength=N_EXP)
    # pair the biggest expert with the smallest per core: slot capacities
    # then shrink to the max of each half instead of the global max
    eorder = np.argsort(-counts, kind="stable")
    big, small = eorder[:N_CORES], eorder[: N_CORES - 1 : -1]
    CA = max(2 * GROUP, int(math.ceil(counts[big].max() / 8) * 8))
    CB = max(2 * GROUP, int(math.ceil(counts[small].max() / 8) * 8))
    nc = _get_program(CA, CB)
    groups = _block_schedule(CA, CB)
    n_groups = len(groups)
    fc2_blocks = _fc2_blocks(n_groups)
    pre = np.concatenate([[0], np.cumsum([n for _, _, _, n in groups])])

    order = np.argsort(idx, kind="stable")
    bounds = np.zeros(N_EXP + 1, dtype=np.int64)
    np.cumsum(counts, out=bounds[1:])
    tok_by_expert = [order[bounds[e] : bounds[e + 1]] for e in range(N_EXP)]

    # shared-layer weights: ws_host[p, hc, kc, m] = (Ws/S)[kc*128+p, hc*128+m]
    ws_host = np.ascontiguousarray(
        (Ws / X_SCALE).reshape(4, 128, 2, 128).transpose(1, 2, 0, 3)
    ).reshape(128, 2 * 4 * 128).astype(BF16)
    bs_host = np.ascontiguousarray(bs.reshape(2, 128).T).astype(np.float32)

    x_q = (x * X_SCALE).astype(F8E3)
    in_maps = []
    core_tokens = []
    for c in range(N_CORES):
        ea, eb = int(big[c]), int(small[c])
        # per-slot token lists padded to capacity with token 0
        toks = np.zeros(CA + CB, dtype=np.int64)
        toks[: counts[ea]] = tok_by_expert[ea]
        toks[CA : CA + counts[eb]] = tok_by_expert[eb]
        core_tokens.append(toks)

        # execution-order groups, column-compacted: group g at cols
        # [4*pre[g], 4*pre[g]+4*n) as [p, kc, t]
        xg = np.empty((128, 4 * (CA + CB)), dtype=F8E3)
        for g, (_, slot, off, n) in enumerate(groups):
            blk = x_q[toks[off : off + n]].reshape(n, 4, 128).transpose(2, 1, 0)
            xg[:, 4 * int(pre[g]) : 4 * int(pre[g]) + 4 * n] = np.ascontiguousarray(
                blk
            ).reshape(128, 4 * n)

        w1_pair = np.zeros((2, 2, 128, 128), dtype=BF16)
        w1_pair[:, :, :, :EXP_HID] = W1[[ea, eb]].reshape(2, 2, 128, EXP_HID).astype(BF16)
        # p-major contiguous: w1_pair[p, e, kc, m]
        w1_pair = np.ascontiguousarray(w1_pair.transpose(2, 0, 1, 3)).reshape(
            128, 2 * 2 * 128
        )
        b1_pair = np.ones((128, 2), dtype=np.float32)
        b1_pair[:EXP_HID] = b1[[ea, eb]].T
        w2_pair = np.zeros((128, 2, 4), dtype=BF16)
        for s_ in range(2):
            col = np.zeros(128, dtype=np.float32)
            col[:EXP_HID] = W2[[ea, eb][s_], :, 0]
            col[EXP_HID] = b2[[ea, eb][s_], 0]
            for lane in range(4):
                w2_pair[:, s_, lane] = col.astype(BF16)
        w2_pair = w2_pair.reshape(128, 2 * 4)

        in_maps.append(
            {
                "xg": xg,
                "ws": ws_host,
                "bs": bs_host,
                "w1": w1_pair,
                "b1": b1_pair,
                "w2c": w2_pair,
            }
        )

    res = run_bass_kernel_spmd(
        nc,
        in_maps,
        core_ids=list(range(N_CORES)),
        trace=_trace,
        **({"trace_cores": [0]} if _trace else {}),
    )
    if _result_box is not None:
        _result_box.append(res)

    out = np.zeros((B, OUT_DIM), dtype=np.float32)
    for c in range(N_CORES):
        ea, eb = int(big[c]), int(small[c])
        oc = res.results[c]["out"]  # [n_blk, 4, GROUP]
        # scatter back: group g's cols [0, n) are slot tokens [off, off+n)
        vals = np.zeros(CA + CB, dtype=np.float32)
        for bi, js in enumerate(fc2_blocks):
            for lane, g in enumerate(js):
                _, slot, off, n = groups[g]
                vals[off : off + n] = oc[bi, lane, :n]
        out[core_tokens[c][: counts[ea]], 0] = vals[: counts[ea]]
        out[core_tokens[c][CA : CA + counts[eb]], 0] = vals[CA : CA + counts[eb]]
    return out


# revision 64
# speedup vs baseline: 1.0502x; 1.0502x over previous
"""MoE routing kernel for 8 Trainium2 NeuronCores.

Problem: B=65536 tokens, shared Linear(512->256)+ReLU, then per-token expert
MLP Linear(256->100)+ReLU -> Linear(100->1), expert chosen by idx in [0,16).

Strategy (expert-parallel, host-side routing):
  - Host sorts tokens by expert. Experts 2c and 2c+1 go to core c, each in a
    fixed-capacity slot (CA/CB = per-half max expert count rounded up to 8),
    padded with token 0 (padding outputs are computed then discarded).
  - x ships as fp8-e3m4 (x*2, with Ws/2 folded on the host): the PE accepts a
    mixed fp8-moving x bf16-stationary matmul at full bf16 rate, so only the
    x DMA bytes halve (rel err ~1.4e-2 vs the 2e-2 gate; fp8 weights or
    e4m3 anywhere fail the gate). xg is column-compacted on the host (each
    exec group packed densely at a prefix-sum offset) so every DMA moves
    contiguous >=512B per-partition rows.
  - Device, per group of <=512 tokens: 8 accumulating layer-1 matmuls
    (512-dim contraction, 2 hid chunks) + bias/ReLU (Vector/Scalar), 2
    matmuls for expert FC1 + bias/ReLU, 1 matmul for FC2 (b2 folded via a
    ones row of h1; stationary zero-padded to 128 cols so the PE tile
    config never changes). FOUR groups' FC2 rows accumulate into one PSUM
    bank at partitions 0/32/64/96, then one copy + one DMA out per block.
  - FOUR-deep software pipeline: iteration i runs L1(i), fc1(i-2), fc2(i-4),
    so every ReLU has ~2 groups of slack before its consuming matmul - one
    group is not enough during the small ramp groups, and any PE gap also
    defers the clock un-throttle (below) by a whole ~3.4us HAM window.
  - Clock: the PE boots throttled to 1.2GHz and un-throttles at a
    free-running ~3.4us HAM-window boundary after a fully-busy window.
    7 warm-up matmuls (never read, uninitialized operands) + the gap-free
    early stream keep it busy from program start; the warm-ups end right as
    the first x lands.
  - DMA: everything bulky rides the sync HWDGE ring in exact need-order
    (ws-hc1, x1, w1a, x2, x3, then PAIRS of groups per post - each post
    costs ~650ns of Sync time, which would otherwise cap the early supply
    rate; w1b drips in late). ONLY ws-hc0 + x0 ride the scalar ring, whose
    first packets arrive ~0.4us earlier - anything later there starves once
    the sync backlog builds. Tiny biases ride the gpsimd SWDGE. A supply
    ramp [128,128,256,256,256] opens slot A so the PE falls safely behind
    the DMA frontier.
  - Tail: slot B is carved so the LAST FOUR groups are [256,256,256,128]
    sharing one PSUM bank: the program ends with one short copy + one small
    DMA + the fixed ~1.6us HBM write receipt, before the NEFF epilogue
    (~6us of fixed ucode semaphore zeroing, outside our control but inside
    the measured window). A small keep-alive DMA keeps the sync ring hot
    for the final post (it also writes warm_w for the allocator).
  - Weights (tiny) live resident in SBUF in bf16; PSUM accumulates fp32.
"""

import math
import os
import sys

import numpy as np

for _p in ("/opt/trn_rl_repo", "/opt/pypackages"):
    if _p not in sys.path and os.path.isdir(_p):
        sys.path.append(_p)

import ml_dtypes

BF16 = ml_dtypes.bfloat16
F8E3 = ml_dtypes.float8_e3m4
X_SCALE = 2.0  # x*2 / Ws/2: shifts small |x| out of the e3m4 subnormal range

B, IN_DIM, HID, EXP_HID, OUT_DIM, N_EXP = 65536, 512, 256, 100, 1, 16
N_CORES = 8
GROUP = 512  # tokens per matmul group (= PSUM bank free-dim in fp32)
N_WARM = 9  # warm-up matmuls (~0.43us each cold), ending ~when the first x
# lands (measured data-gate ~11.2-11.6us from program start). The PE-clock HAM un-throttles at a free-running ~3.4us window
# boundary once it sees a fully-busy window: warm-ups + a GAP-FREE early
# real stream together cover the worst-case ~6.8us, so the clock warms as
# early as the phase allows (any early gap defers it by a whole window).

_PROGRAM_CACHE = {}


def _block_schedule(CA: int, CB: int):
    """Execution-order blocks: (exec_idx, expert_slot, token_offset, ntok).

    Slot A opens with a supply ramp over its first 1024 tokens (the DMA
    backlog is still building; the PE must fall safely behind the DMA
    frontier before group sizes reach 512 so the early stream is GAP-FREE
    for the clock-warming HAM window), then its full groups and remainder;
    slot B's full groups follow, with its tail carved into FOUR final
    groups [p, q, r, 128] that form the last 4-lane fc2 block (one copy +
    one small DMA at program end)."""
    n_full_b, r_b = divmod(CB, GROUP)
    if CA >= 2048:
        # = 4 full groups of ramp: PE demand stays ~1us behind the DMA
        # frontier through the whole clock-warming window
        a = [128, 128, 256, 256, 256, 384, 384, 256]
        rem = CA - 2048
    else:
        a = [128, 128, 256, 256, 256]
        rem = CA - 1024
    a += [GROUP] * (rem // GROUP)
    if rem % GROUP:
        a.append(rem % GROUP)
    b = [GROUP] * n_full_b
    if r_b:
        b.append(r_b)
    # carve the tail so the final four groups are ALWAYS [256, 256, 256, 128]
    # (the last 4-lane fc2 block: one short copy + one small DMA at program
    # end); the carve overflow becomes a regular group before them
    T = 0
    while b and T < 896:
        T += b.pop()
    if T > 896:
        b.append(T - 896)
    b += [256, 256, 256, 128]
    blocks = [(0, n) for n in a] + [(1, n) for n in b]
    out = []
    off = [0, 0]
    base = [0, CA]
    for slot, n in blocks:
        out.append((len(out), slot, base[slot] + off[slot], n))
        off[slot] += n
    return out


def _fc2_blocks(n_groups: int):
    """fc2 PSUM-sharing blocks: chunks of FOUR exec-groups (output lanes at
    partitions 0/32/64/96 of one PSUM bank — the 128-wide zero-padded
    stationary makes a 4th lane free). Built from the END so the final
    block is exactly the four small carved tail groups: the program ends
    with one (engine-split) copy + ONE small DMA; the HBM write receipt
    dominates the close."""
    head = n_groups - 4
    first = head % 4
    blocks = [list(range(first))] if first else []
    blocks += [list(range(i, i + 4)) for i in range(first, head, 4)]
    blocks.append([head, head + 1, head + 2, head + 3])
    return blocks


def _build_program(CA: int, CB: int):
    """Build (and cache) the Bass program for slot capacities (CA, CB)."""
    import concourse.bass as bass
    import concourse.mybir as mybir
    import concourse.tile as tile
    from concourse import bacc

    f32 = mybir.dt.float32
    bf16 = mybir.dt.bfloat16
    f8e3 = mybir.dt.float8e3
    AF = mybir.ActivationFunctionType
    ALU = mybir.AluOpType

    nc = bacc.Bacc("TRN2", target_bir_lowering=False, debug=False)

    groups = _block_schedule(CA, CB)
    n_groups = len(groups)
    fc2_blocks = _fc2_blocks(n_groups)
    n_blk = len(fc2_blocks)
    blk_of = {}
    blk_max = {}
    for bi, js in enumerate(fc2_blocks):
        blk_max[bi] = max(groups[j][3] for j in js)
        for lane, j in enumerate(js):
            blk_of[j] = (bi, lane, lane == len(js) - 1)

    # x pre-gathered on host in EXECUTION order, column-compacted fp8:
    # group g occupies cols [4*pre[g], 4*pre[g]+4*n) as [p, kc, t]
    pre = np.concatenate([[0], np.cumsum([n for _, _, _, n in groups])])
    tot_cols = 4 * int(pre[-1])
    xg_d = nc.dram_tensor("xg", [128, tot_cols], f8e3, kind="ExternalInput").ap()
    # ws packed p-major with contiguous hc halves: ws[p, hc, kc, m]
    ws_d = nc.dram_tensor("ws", [128, 2 * 4 * 128], bf16, kind="ExternalInput").ap()
    bs_d = nc.dram_tensor("bs", [128, 2], f32, kind="ExternalInput").ap()
    # w1 padded to 128 output cols (100 real) so FWL kicks in on LDWEIGHTS
    w1_d = nc.dram_tensor("w1", [128, 2 * 2 * 128], bf16, kind="ExternalInput").ap()
    # only the real fc2 columns come from DRAM (2KB); the mostly-zero
    # 128-wide stationary tiles are assembled on-chip
    w2c_d = nc.dram_tensor("w2c", [128, 2 * 4], bf16, kind="ExternalInput").ap()
    # b1 rows 0..99 = b1[e]; rows 100..127 = 1.0 so relu(0 + 1) makes a ones
    # row block that w2's bias row consumes (fc2 bias folded into the matmul)
    b1_d = nc.dram_tensor("b1", [128, 2], f32, kind="ExternalInput").ap()
    # out[blk, lane, t]: lanes hold the block's exec-groups' fc2 rows
    out_d = nc.dram_tensor("out", [n_blk, 4, GROUP], f32, kind="ExternalOutput").ap()

    with tile.TileContext(nc) as tc:
        with (
            tc.tile_pool(name="const", bufs=1) as const,
            tc.tile_pool(name="xp", bufs=1) as xp,
            tc.tile_pool(name="hp", bufs=5) as hp,
            tc.tile_pool(name="h1p", bufs=5) as h1p,
            tc.tile_pool(name="ob", bufs=3) as obp,
            tc.tile_pool(name="ps1", bufs=4, space="PSUM") as ps1,
            tc.tile_pool(name="ps2", bufs=2, space="PSUM") as ps2,
            tc.tile_pool(name="ps3", bufs=2, space="PSUM") as ps3,
        ):
            ws_sb = const.tile([128, 2, 4, 128], bf16)
            bs_sb = const.tile([128, 2], f32)
            w1_sb = const.tile([128, 2, 2, 128], bf16)
            b1_sb = const.tile([128, 2], f32)
            w2_sb = const.tile([128, 2, 4, 128], bf16)
            w2c_sb = const.tile([128, 2, 4], bf16)
            warm_w = const.tile([128, GROUP], f8e3)
            # x tiles are FLAT, dense, matching the DRAM layout, so src AND
            # dst of every x DMA are contiguous per-partition rows (>=512B
            # packets; a strided dst would split partial groups into
            # sub-512B packets, ~4x slower). From group 4 on, TWO adjacent
            # groups share one tile and ship in ONE DMA post: each post
            # costs ~650ns of Sync time, so per-group posting would cap the
            # supply rate (~260KB/650ns ~ 400GB/s) during the early build.
            # L1 slices kc chunks as [:, base + kc*n : base + (kc+1)*n].
            x_tiles = {}  # group -> (tile, base_col)
            post_units = []  # (first_group, src_lo, src_hi, tile, width)
            gi = 0
            while gi < n_groups:
                n1 = groups[gi][3]
                if gi < 4 or gi == n_groups - 1:
                    t = xp.tile([128, 4 * n1], f8e3, tag=f"x{gi}", name=f"x_sb{gi}")
                    x_tiles[gi] = (t, 0)
                    post_units.append((gi, 4 * int(pre[gi]), 4 * int(pre[gi + 1]), t))
                    gi += 1
                else:
                    n2 = groups[gi + 1][3]
                    t = xp.tile(
                        [128, 4 * (n1 + n2)], f8e3, tag=f"x{gi}", name=f"x_sb{gi}"
                    )
                    x_tiles[gi] = (t, 0)
                    x_tiles[gi + 1] = (t, 4 * n1)
                    post_units.append((gi, 4 * int(pre[gi]), 4 * int(pre[gi + 2]), t))
                    gi += 2

            unit_of = {u[0]: k for k, u in enumerate(post_units)}

            def post_x(u, eng=None):
                g0, lo, hi, t = post_units[u]
                (eng or nc.sync).dma_start(t[:, : hi - lo], xg_d[:, lo:hi])

            # dependency-free memset first: it runs during the framework
            # preamble window, long before the first ReLU needs Vector
            nc.vector.memset(w2_sb.rearrange("p e l m -> p (e l m)"), 0.0)

            # EVERYTHING bulky rides the sync ring in exact need-order (two
            # rings starve each other: once the sync ring's packet backlog
            # builds, the scalar ring drops to ~60GB/s and whatever is on it
            # arrives microseconds late). Tiny biases ride the gpsimd SWDGE.
            ws_dv = ws_d.rearrange("p (h k m) -> p h k m", h=2, k=4)
            w1_dv = w1_d.rearrange("p (e x) -> p e x", e=2)
            # the two transfers that gate the FIRST real matmul ride the
            # scalar ring: the Scalar engine exits the preamble ~0.4us
            # before Sync, so its ring spins up earlier — and both complete
            # before the sync ring's backlog could starve the scalar ring
            # only these two ride the scalar ring: they complete BEFORE the
            # sync ring's backlog builds — anything later on the scalar ring
            # starves (SDMA round-robin favors the deep sync ring)
            nc.scalar.dma_start(ws_sb[:, 0], ws_dv[:, 0])
            post_x(0, eng=nc.scalar)
            nc.sync.dma_start(ws_sb[:, 1], ws_dv[:, 1])
            post_x(1)
            nc.sync.dma_start(w1_sb[:, 0:1, :, :], w1_dv[:, 0:1, :])
            nc.gpsimd.dma_start(bs_sb[:, :], bs_d[:, :])
            nc.gpsimd.dma_start(b1_sb[:, :], b1_d[:, :])
            nc.gpsimd.dma_start(w2c_sb[:, :, :], w2c_d)
            # slot B's fc1 weights are first needed when slot B's first group
            # reaches fc1 — keep them out of the contested early window
            first_b = next(g for g, (_, s, _, _) in enumerate(groups) if s == 1)
            w1b_unit = unit_of[min(8, first_b - 1)] if min(8, first_b - 1) in unit_of \
                else unit_of[min(8, first_b - 1) - 1]
            for u in range(2, len(post_units)):
                post_x(u)
                if u == w1b_unit:
                    nc.sync.dma_start(w1_sb[:, 1:2, :, :], w1_dv[:, 1:2, :])

            # PE warm-up: full-width matmuls that keep the PE continuously
            # busy from program start until the first x tile lands, so the
            # PE clock (p-state) is fully ramped when real work begins. An
            # idle PE drops to half speed. Results are never read; warm_w is
            # deliberately uninitialized (garbage is harmless and skipping
            # the memset removes any vector-engine dependency).
            warm_p = ps1.tile([128, GROUP], f32, tag="p1", name="warm_p")
            for _ in range(N_WARM):
                nc.tensor.matmul(
                    warm_p[:, :], warm_w[:, :128], warm_w[:, :], start=True, stop=True
                )

            # assemble the zero-padded 128-wide fc2 stationary tiles: only
            # expert A's lanes are needed for the first fc2 blocks — its
            # copies go here; expert B's are deferred into the pipeline so
            # Scalar is free for the first hc1-ReLUs (the memset was issued
            # at body start so Vector is free too)
            def w2_assemble(e_):
                for lane_ in range(4):
                    nc.scalar.copy(
                        w2_sb[:, e_, lane_, 32 * lane_ : 32 * lane_ + 1],
                        w2c_sb[:, e_, lane_ : lane_ + 1],
                    )

            w2_assemble(0)
            w2_assemble(1)

            # Two-deep software pipeline over the PE stream: iteration i runs
            # L1(i), fc1(i-1), fc2(i-2). Every activation (h, h1) then has
            # >=1.5us between its producing ReLU and its consuming matmul, so
            # the in-order PE stream never stalls on the Vector/Scalar
            # engines.
            h_tiles = {}
            h1_tiles = {}
            p3 = None

            def do_l1(i):
                _, _, _, n = groups[i]
                x_sb, xb = x_tiles[i]
                h_sb = hp.tile([128, 2, GROUP], bf16, tag="h")
                for hc in range(2):
                    p1 = ps1.tile([128, GROUP], f32, tag="p1")
                    for kc in range(4):
                        nc.tensor.matmul(
                            p1[:, :n],
                            ws_sb[:, hc, kc, :],
                            x_sb[:, xb + kc * n : xb + (kc + 1) * n],
                            start=(kc == 0),
                            stop=(kc == 3),
                        )
                    # h = relu(psum + bs): hc0 on VectorE, hc1 on ScalarE
                    if hc == 0:
                        nc.vector.tensor_scalar(
                            h_sb[:, hc, :n],
                            p1[:, :n],
                            bs_sb[:, hc : hc + 1],
                            0.0,
                            ALU.add,
                            ALU.max,
                        )
                    else:
                        nc.scalar.activation(
                            h_sb[:, hc, :n],
                            p1[:, :n],
                            AF.Relu,
                            bias=bs_sb[:, hc : hc + 1],
                        )
                h_tiles[i] = h_sb

            def do_fc1(j):
                if j < 0 or j >= n_groups:
                    return
                _, e, _, n = groups[j]
                h_sb = h_tiles.pop(j)
                p2 = ps2.tile([128, GROUP], f32, tag="p2")
                for kc in range(2):
                    nc.tensor.matmul(
                        p2[:, :n],
                        w1_sb[:, e, kc, :],
                        h_sb[:, kc, :n],
                        start=(kc == 0),
                        stop=(kc == 1),
                    )
                # h1 rows 0..99 = relu(psum + b1); rows 100..127 = relu(0+1)=1
                # alternate engines so neither Vector nor Scalar is loaded 2x
                h1_sb = h1p.tile([128, GROUP], bf16, tag="h1")
                if j % 2 == 0:
                    nc.vector.tensor_scalar(
                        h1_sb[:, :n],
                        p2[:, :n],
                        b1_sb[:, e : e + 1],
                        0.0,
                        ALU.add,
                        ALU.max,
                    )
                else:
                    nc.scalar.activation(
                        h1_sb[:, :n],
                        p2[:, :n],
                        AF.Relu,
                        bias=b1_sb[:, e : e + 1],
                    )
                h1_tiles[j] = h1_sb

            def issue_fc2(j):
                nonlocal p3
                if j < 0 or j >= n_groups:
                    return
                _, e, _, n = groups[j]
                # up to 3 exec-groups share one PSUM bank at partitions
                # 0/32/64 (PE col-tile base must be 0/32/64)
                blk, lane, last = blk_of[j]
                if lane == 0:
                    p3 = ps3.tile([128, GROUP], f32, tag="p3")
                nc.tensor.matmul(
                    p3[:, :n],
                    w2_sb[:, e, lane, :],
                    h1_tiles.pop(j)[:, :n],
                    start=(lane == 0),
                    stop=last,
                )
                if last:
                    ob = obp.tile([128, GROUP], f32, tag="ob")
                    nrows = 32 * lane + 1
                    ncols = blk_max[blk]
                    if blk % 2 == 0:
                        nc.scalar.copy(ob[:nrows, :ncols], p3[:nrows, :ncols])
                    else:
                        nc.vector.tensor_scalar(
                            ob[:nrows, :ncols],
                            p3[:nrows, :ncols],
                            0.0,
                            0.0,
                            ALU.add,
                            ALU.bypass,
                        )
                    # rows 0/32/64 of ob hold the lanes' fc2 outputs
                    ob_lanes = ob.rearrange("(l s) t -> l s t", s=32)[:, 0, :]
                    nc.sync.dma_start(
                        out_d[blk, : lane + 1, :ncols], ob_lanes[: lane + 1, :ncols]
                    )

            # FOUR-deep software pipeline: iteration i runs L1(i), fc1(i-2),
            # fc2(i-4): every ReLU gets ~2 iterations of slack before its
            # consuming matmul (1 iteration is not enough during the small
            # ramp groups, where an h/h1 stall also re-throttles the PE clock)
            for i in range(n_groups):
                do_l1(i)
                do_fc1(i - 2)
                issue_fc2(i - 4)
            do_fc1(n_groups - 2)
            do_fc1(n_groups - 1)
            issue_fc2(n_groups - 4)
            issue_fc2(n_groups - 3)
            # small keep-alive so the sync ring is hot for the final posts
            # (also the write that lets the allocator place warm_w); it sits
            # one slot earlier in the ring FIFO so the final out post's path
            # is clean
            nc.sync.dma_start(warm_w[:, :], xg_d[:, :GROUP])
            issue_fc2(n_groups - 2)
            issue_fc2(n_groups - 1)

    nc.compile()
    return nc


def _get_program(CA: int, CB: int):
    if (CA, CB) not in _PROGRAM_CACHE:
        _PROGRAM_CACHE[(CA, CB)] = _build_program(CA, CB)
    return _PROGRAM_CACHE[(CA, CB)]


def kernel(x, idx, Ws, bs, W1, b1, W2, b2, _trace=False, _result_box=None):
    from concourse.bass_utils import run_bass_kernel_spmd

    x = np.asarray(x)
    idx = np.asarray(idx).astype(np.int64)
    Ws = np.asarray(Ws, dtype=np.float32)
    bs = np.asarray(bs, dtype=np.float32)
    W1 = np.asarray(W1, dtype=np.float32)
    b1 = np.asarray(b1, dtype=np.float32)
    W2 = np.asarray(W2, dtype=np.float32)
    b2 = np.asarray(b2, dtype=np.float32)

    counts = np.bincount(idx, minlength=N_EXP)
    # pair the biggest expert with the smallest per core: slot capacities
    # then shrink to the max of each half instead of the global max
    eorder = np.argsort(-counts, kind="stable")
    big, small = eorder[:N_CORES], eorder[: N_CORES - 1 : -1]
    CA = max(2 * GROUP, int(math.ceil(counts[big].max() / 8) * 8))
    CB = max(2 * GROUP, int(math.ceil(counts[small].max() / 8) * 8))
    nc = _get_program(CA, CB)
    groups = _block_schedule(CA, CB)
    n_groups = len(groups)
    fc2_blocks = _fc2_blocks(n_groups)
    pre = np.concatenate([[0], np.cumsum([n for _, _, _, n in groups])])

    order = np.argsort(idx, kind="stable")
    bounds = np.zeros(N_EXP + 1, dtype=np.int64)
    np.cumsum(counts, out=bounds[1:])
    tok_by_expert = [order[bounds[e] : bounds[e + 1]] for e in range(N_EXP)]

    # shared-layer weights: ws_host[p, hc, kc, m] = (Ws/S)[kc*128+p, hc*128+m]
    ws_host = np.ascontiguousarray(
        (Ws / X_SCALE).reshape(4, 128, 2, 128).transpose(1, 2, 0, 3)
    ).reshape(128, 2 * 4 * 128).astype(BF16)
    bs_host = np.ascontiguousarray(bs.reshape(2, 128).T).astype(np.float32)

    x_q = (x * X_SCALE).astype(F8E3)
    in_maps = []
    core_tokens = []
    for c in range(N_CORES):
        ea, eb = int(big[c]), int(small[c])
        # per-slot token lists padded to capacity with token 0
        toks = np.zeros(CA + CB, dtype=np.int64)
        toks[: counts[ea]] = tok_by_expert[ea]
        toks[CA : CA + counts[eb]] = tok_by_expert[eb]
        core_tokens.append(toks)

        # execution-order groups, column-compacted: group g at cols
        # [4*pre[g], 4*pre[g]+4*n) as [p, kc, t]
        xg = np.empty((128, 4 * (CA + CB)), dtype=F8E3)
        for g, (_, slot, off, n) in enumerate(groups):
            blk = x_q[toks[off : off + n]].reshape(n, 4, 128).transpose(2, 1, 0)
            xg[:, 4 * int(pre[g]) : 4 * int(pre[g]) + 4 * n] = np.ascontiguousarray(
                blk
            ).reshape(128, 4 * n)

        w1_pair = np.zeros((2, 2, 128, 128), dtype=BF16)
        w1_pair[:, :, :, :EXP_HID] = W1[[ea, eb]].reshape(2, 2, 128, EXP_HID).astype(BF16)
        # p-major contiguous: w1_pair[p, e, kc, m]
        w1_pair = np.ascontiguousarray(w1_pair.transpose(2, 0, 1, 3)).reshape(
            128, 2 * 2 * 128
        )
        b1_pair = np.ones((128, 2), dtype=np.float32)
        b1_pair[:EXP_HID] = b1[[ea, eb]].T
        w2_pair = np.zeros((128, 2, 4), dtype=BF16)
        for s_ in range(2):
            col = np.zeros(128, dtype=np.float32)
            col[:EXP_HID] = W2[[ea, eb][s_], :, 0]
            col[EXP_HID] = b2[[ea, eb][s_], 0]
            for lane in range(4):
                w2_pair[:, s_, lane] = col.astype(BF16)
        w2_pair = w2_pair.reshape(128, 2 * 4)

        in_maps.append(
            {
                "xg": xg,
                "ws": ws_host,
                "bs": bs_host,
                "w1": w1_pair,
                "b1": b1_pair,
                "w2c": w2_pair,
            }
        )

    res = run_bass_kernel_spmd(
        nc,
        in_maps,
        core_ids=list(range(N_CORES)),
        trace=_trace,
        **({"trace_cores": [0]} if _trace else {}),
    )
    if _result_box is not None:
        _result_box.append(res)

    out = np.zeros((B, OUT_DIM), dtype=np.float32)
    for c in range(N_CORES):
        ea, eb = int(big[c]), int(small[c])
        oc = res.results[c]["out"]  # [n_blk, 4, GROUP]
        # scatter back: group g's cols [0, n) are slot tokens [off, off+n)
        vals = np.zeros(CA + CB, dtype=np.float32)
        for bi, js in enumerate(fc2_blocks):
            for lane, g in enumerate(js):
                _, slot, off, n = groups[g]
                vals[off : off + n] = oc[bi, lane, :n]
        out[core_tokens[c][: counts[ea]], 0] = vals[: counts[ea]]
        out[core_tokens[c][CA : CA + counts[eb]], 0] = vals[CA : CA + counts[eb]]
    return out
